# revision 12
# baseline (speedup 1.0000x reference)
"""Trainium2 Bass kernel for LSTNet-style model (conv -> band-sum -> GRU ->
skip-GRU -> linear + highway -> sigmoid), SPMD across 8 NeuronCores.

Sharding: conv GEMM is tensor-parallel over output channels (4 of 32 channels
per core, full batch B=128); an AllToAll then re-shards to data-parallel
(B/8 = 16 batch rows per core) for the recurrent + output stages.
"""

from contextlib import ExitStack

import numpy as np

import concourse.bass as bass
import concourse.mybir as mybir
import concourse.tile as tile
from concourse import bacc
from concourse.bass_utils import run_bass_kernel_spmd

F32 = mybir.dt.float32
AF = mybir.ActivationFunctionType

# Model hyperparameters (hardcoded; must match reference.py)
P = 168
M1, M2, M3 = 2, 3, 4
HIDC = 32
CK = 6
HIDR = 100
HIDS = 5
SKIP = 24
HWIN = 24
B = 128
L = P - CK + 1          # 163
PT = 6
M = M1 * M2 * M3        # 24
NC = 8                  # cores
KC = P * M1 * M2 * M3   # 4032 conv contraction
KCP = 4096              # padded contraction (row 4032 = ones/bias row)
OC = P * HIDC * CK      # 32256 conv outputs
OPC = OC // NC          # 4032 outputs per core (4 channels)
CHC = HIDC // NC        # 4 channels per core
BL = B // NC            # 16 batch rows per core after A2A
NG = (L + 7) // 8       # 21 groups of (up to) 8 GRU steps


def build_program():
    nc = bacc.Bacc(
        "TRN2",
        target_bir_lowering=False,
        debug=False,
        enable_asserts=True,
        num_devices=NC,
    )

    # ---- kernel I/O ----
    def din(name, shape):
        return nc.dram_tensor(name, list(shape), F32, kind="ExternalInput").ap()

    def dout(name, shape):
        return nc.dram_tensor(name, list(shape), F32, kind="ExternalOutput").ap()

    xt_d = din("xt", [128, KCP])            # x^T tiled [kin, (ktile, b)]
    wt_d = din("wt", [64, 128, 2016])       # conv W^T tiles (per-core slice)
    whr_d = din("whr", [HIDR + 1, HIDR])
    whz_d = din("whz", [HIDR + 1, HIDR])
    whn_d = din("whn", [HIDR + 1, HIDR])
    wxr_d = din("wxr", [HIDC + 1, HIDR])
    wxz_d = din("wxz", [HIDC + 1, HIDR])
    wxn_d = din("wxn", [HIDC + 1, HIDR])
    lsr_d = din("lsr", [121, 120])
    lsz_d = din("lsz", [121, 120])
    lsn_d = din("lsn", [121, 120])
    wxs_d = din("wxs", [HIDC + 1, 15])
    w2a_d = din("w2a", [HIDR, M])
    w2b_d = din("w2b", [121, M])
    xh_d = din("xh", [HWIN + 1, M * BL])    # highway lhsT (per-core b-slice)
    hwv_d = din("hwv", [HWIN + 1, 1])
    hinit_d = din("hinit", [HIDR + 1, BL])
    hsinit_d = din("hsinit", [121, BL])

    out_d = dout("out", [BL, M])
    dbg_cc_d = dout("dbg_cc", [B, CHC * L])
    dbg_c_d = dout("dbg_c", [HIDC + 1, L * BL])
    dbg_h_d = dout("dbg_h", [HIDR, BL])
    dbg_hs_d = dout("dbg_hs", [121, BL])

    with tile.TileContext(nc) as tc, ExitStack() as ctx:
        consts = ctx.enter_context(tc.tile_pool(name="consts", bufs=1))

        # ---- load all small weights ----
        xt_sb = consts.tile([128, KCP], F32)
        nc.sync.dma_start(xt_sb[:], xt_d)
        whr = consts.tile([HIDR + 1, HIDR], F32)
        whz = consts.tile([HIDR + 1, HIDR], F32)
        whn = consts.tile([HIDR + 1, HIDR], F32)
        wxr = consts.tile([HIDC + 1, HIDR], F32)
        wxz = consts.tile([HIDC + 1, HIDR], F32)
        wxn = consts.tile([HIDC + 1, HIDR], F32)
        nc.sync.dma_start(whr[:], whr_d)
        nc.sync.dma_start(whz[:], whz_d)
        nc.sync.dma_start(whn[:], whn_d)
        nc.sync.dma_start(wxr[:], wxr_d)
        nc.sync.dma_start(wxz[:], wxz_d)
        nc.sync.dma_start(wxn[:], wxn_d)
        lsr = consts.tile([121, 120], F32)
        lsz = consts.tile([121, 120], F32)
        lsn = consts.tile([121, 120], F32)
        wxs = consts.tile([HIDC + 1, 15], F32)
        nc.sync.dma_start(lsr[:], lsr_d)
        nc.sync.dma_start(lsz[:], lsz_d)
        nc.sync.dma_start(lsn[:], lsn_d)
        nc.sync.dma_start(wxs[:], wxs_d)
        w2a = consts.tile([HIDR, M], F32)
        w2b = consts.tile([121, M], F32)
        xh = consts.tile([HWIN + 1, M * BL], F32)
        hwv = consts.tile([HWIN + 1, 1], F32)
        nc.sync.dma_start(w2a[:], w2a_d)
        nc.sync.dma_start(w2b[:], w2b_d)
        nc.sync.dma_start(xh[:], xh_d)
        nc.sync.dma_start(hwv[:], hwv_d)

        crelu = consts.tile([B, OPC], F32)      # conv output (post relu)
        cc = consts.tile([B, CHC * L], F32)     # band-summed [b, (ch, t)]
        fpsum = ctx.enter_context(tc.tile_pool(name="fpsum", bufs=1, space="PSUM"))
        ps_fin = fpsum.tile([BL, 32], F32)

        # =========== Stage 1: conv GEMM (output-channel sharded) ===========
        with (
            tc.tile_pool(name="wpool", bufs=3) as wpool,
            tc.tile_pool(name="cpsum", bufs=2, space="PSUM") as cpsum,
        ):
            OCW = 504
            for oc in range(8):
                ps = cpsum.tile([128, OCW], F32)
                for kg in range(8):
                    wtile = wpool.tile([128, 4 * OCW], F32, tag="wtile")
                    nc.sync.dma_start(wtile[:], wt_d[oc * 8 + kg])
                    for kl in range(4):
                        kt = 4 * kg + kl
                        nc.tensor.matmul(
                            ps[:, :],
                            xt_sb[:, kt * 128:(kt + 1) * 128],
                            wtile[:, kl * OCW:(kl + 1) * OCW],
                            start=(kg == 0 and kl == 0),
                            stop=(kg == 7 and kl == 3),
                        )
                # ReLU: psum -> sbuf
                nc.scalar.activation(
                    crelu[:, oc * OCW:(oc + 1) * OCW], ps[:, :], AF.Relu
                )

            # ---- highway matmuls (independent of conv; fill PE idle) ----
            for m in range(M):
                nc.tensor.matmul(
                    ps_fin[:, m:m + 1],
                    xh[:, m * BL:(m + 1) * BL],
                    hwv[:, :],
                    start=(m == 0),
                    stop=False,
                    skip_group_check=True,
                )

            # ---- band sum: cc[b, ch, t] = sum_k crelu[b, ch*1008 + 169k + t]
            for ch in range(CHC):
                base = ch * (CK * P)
                dst = cc[:, ch * L:(ch + 1) * L]
                nc.vector.tensor_add(
                    dst, crelu[:, base:base + L], crelu[:, base + 169:base + 169 + L]
                )
                for k in range(2, CK):
                    nc.vector.tensor_add(
                        dst, dst, crelu[:, base + 169 * k:base + 169 * k + L]
                    )

        # =========== Stage 2: AllToAll (channel-shard -> batch-shard) =======
        dram = ctx.enter_context(tc.tile_pool(name="dram", bufs=1, space="DRAM"))
        cc_d = dram.tile([B, CHC * L], F32)
        cta_d = dram.tile([B, CHC * L], F32)
        nc.sync.dma_start(cc_d[:], cc[:])
        nc.gpsimd.collective_compute(
            "AllToAll",
            mybir.AluOpType.bypass,
            replica_groups=[list(range(NC))],
            ins=[cc_d.opt()],
            outs=[cta_d.opt()],
        )
        nc.sync.dma_start(dbg_cc_d, cc[:])

        # gather into SBUF as c_aug [33, (t, b)] : rows = 32 channels + ones
        c_aug = consts.tile([HIDC + 1, L * BL], F32)
        # cta_d rows: (rank r, bb 16) ; cols: (ch_local 4, t 163)
        for r in range(NC):
            src_r = cta_d[r * BL:(r + 1) * BL, :].rearrange(
                "bb (c t) -> c t bb", c=CHC
            )  # [4, 163, 16]
            dst_r = c_aug[r * CHC:(r + 1) * CHC, :].rearrange(
                "c (t bb) -> c t bb", bb=BL
            )
            nc.sync.dma_start(dst_r, src_r)
        nc.gpsimd.memset(c_aug[HIDC:HIDC + 1, :], 1.0)
        nc.sync.dma_start(dbg_c_d, c_aug[:])

        # =========== Stage 3: skip-GRU x-side projections (hoisted) =========
        # gis_g[(k,i)=120, (p, b)=96] for gate g in (r, z, n)
        gis_r = consts.tile([120, PT * BL], F32)
        gis_z = consts.tile([120, PT * BL], F32)
        gis_n = consts.tile([120, PT * BL], F32)
        c3 = c_aug[:].rearrange("p (t bb) -> p t bb", bb=BL)  # [33, 163, 16]
        with (
            tc.tile_pool(name="ppsum", bufs=2, space="PSUM") as ppsum,
            tc.tile_pool(name="pstage", bufs=2) as pstage,
        ):
            for k in range(SKIP):
                psk = ppsum.tile([15, PT * BL], F32)
                rhs = c3[:, L - PT * SKIP + k:L:SKIP, :]  # [33, 6, 16]
                nc.tensor.matmul(
                    psk[:, :].rearrange("p (t bb) -> p t bb", bb=BL),
                    wxs[:, :],
                    rhs,
                    start=True,
                    stop=True,
                )
                stg = pstage.tile([15, PT * BL], F32, tag="stg")
                nc.scalar.activation(stg[:, :], psk[:, :], AF.Copy)
                nc.sync.dma_start(gis_r[5 * k:5 * k + 5, :], stg[0:5, :])
                nc.sync.dma_start(gis_z[5 * k:5 * k + 5, :], stg[5:10, :])
                nc.sync.dma_start(gis_n[5 * k:5 * k + 5, :], stg[10:15, :])

            # =========== Stage 4: main GRU over 163 steps ===========
            h_aug = consts.tile([HIDR + 1, BL], F32)
            nc.sync.dma_start(h_aug[:], hinit_d)

            with (
                tc.tile_pool(name="gpsum", bufs=3, space="PSUM") as gpsum,
                tc.tile_pool(name="gwork", bufs=4) as gwork,
            ):
                for g in range(NG):
                    ns = min(8, L - 8 * g)        # steps in this group
                    nb = ns * BL                  # columns of x-projections
                    ps = gpsum.tile([128, 512], F32, tag="ps", bufs=3)
                    cslice = c_aug[:, g * 8 * BL:g * 8 * BL + nb]
                    # x-side projections for the whole group
                    nc.tensor.matmul(ps[0:HIDR, 0:nb], wxr[:, :], cslice,
                                     start=True, stop=False,
                                     skip_group_check=True)
                    nc.tensor.matmul(ps[0:HIDR, 128:128 + nb], wxz[:, :], cslice,
                                     start=False, stop=False,
                                     skip_group_check=True)
                    nc.tensor.matmul(ps[0:HIDR, 384:384 + nb], wxn[:, :], cslice,
                                     start=False, stop=False,
                                     skip_group_check=True)
                    for s in range(ns):
                        o = s * BL
                        # hidden-side matmuls (accumulate r/z; fresh hn)
                        nc.tensor.matmul(ps[0:HIDR, o:o + BL], whr[:, :],
                                         h_aug[:, :], start=False, stop=False,
                                         skip_group_check=True)
                        nc.tensor.matmul(ps[0:HIDR, 128 + o:128 + o + BL],
                                         whz[:, :], h_aug[:, :],
                                         start=False, stop=False,
                                         skip_group_check=True)
                        nc.tensor.matmul(ps[0:HIDR, 256 + o:256 + o + BL],
                                         whn[:, :], h_aug[:, :],
                                         start=False, stop=(s == ns - 1),
                                         skip_group_check=True)
                        rz = gwork.tile([HIDR, 2 * BL], F32, tag="rz")
                        psv = ps[:].rearrange("p (q f) -> p q f", q=4)
                        nc.scalar.activation(
                            rz[:, :].rearrange("p (q f) -> p q f", q=2),
                            psv[0:HIDR, 0:2, o:o + BL],
                            AF.Sigmoid,
                        )
                        t1 = gwork.tile([HIDR, BL], F32, tag="t1")
                        nc.vector.tensor_mul(
                            t1[:, :], rz[:, 0:BL], ps[0:HIDR, 256 + o:256 + o + BL]
                        )
                        t2 = gwork.tile([HIDR, BL], F32, tag="t2")
                        nc.vector.tensor_add(
                            t2[:, :], t1[:, :], ps[0:HIDR, 384 + o:384 + o + BL]
                        )
                        n_t = gwork.tile([HIDR, BL], F32, tag="n_t")
                        nc.scalar.activation(n_t[:, :], t2[:, :], AF.Tanh)
                        d_t = gwork.tile([HIDR, BL], F32, tag="d_t")
                        nc.vector.tensor_sub(d_t[:, :], h_aug[0:HIDR, :], n_t[:, :])
                        e_t = gwork.tile([HIDR, BL], F32, tag="e_t")
                        nc.vector.tensor_mul(e_t[:, :], rz[:, BL:2 * BL], d_t[:, :])
                        nc.vector.tensor_add(h_aug[0:HIDR, :], n_t[:, :], e_t[:, :])

                nc.sync.dma_start(dbg_h_d, h_aug[0:HIDR, :])

                # =========== Stage 5: skip-GRU (6 steps) ===========
                hs_aug = consts.tile([121, BL], F32)
                nc.sync.dma_start(hs_aug[:], hsinit_d)
                for p in range(PT):
                    pss = gpsum.tile([128, 512], F32, tag="pss", bufs=2)
                    nc.tensor.matmul(pss[0:120, 0:BL], lsr[:, :], hs_aug[:, :],
                                     start=True, stop=False,
                                     skip_group_check=True)
                    nc.tensor.matmul(pss[0:120, BL:2 * BL], lsz[:, :],
                                     hs_aug[:, :], start=False, stop=False,
                                     skip_group_check=True)
                    nc.tensor.matmul(pss[0:120, 2 * BL:3 * BL], lsn[:, :],
                                     hs_aug[:, :], start=False, stop=True,
                                     skip_group_check=True)
                    o = p * BL
                    arz = gwork.tile([120, 2 * BL], F32, tag="arz")
                    nc.vector.tensor_add(
                        arz[:, 0:BL], pss[0:120, 0:BL], gis_r[:, o:o + BL]
                    )
                    nc.vector.tensor_add(
                        arz[:, BL:2 * BL], pss[0:120, BL:2 * BL],
                        gis_z[:, o:o + BL]
                    )
                    rzs = gwork.tile([120, 2 * BL], F32, tag="rzs")
                    nc.scalar.activation(rzs[:, :], arz[:, :], AF.Sigmoid)
                    t1s = gwork.tile([120, BL], F32, tag="t1s")
                    nc.vector.tensor_mul(
                        t1s[:, :], rzs[:, 0:BL], pss[0:120, 2 * BL:3 * BL]
                    )
                    t2s = gwork.tile([120, BL], F32, tag="t2s")
                    nc.vector.tensor_add(t2s[:, :], t1s[:, :], gis_n[:, o:o + BL])
                    ns_t = gwork.tile([120, BL], F32, tag="ns_t")
                    nc.scalar.activation(ns_t[:, :], t2s[:, :], AF.Tanh)
                    ds_t = gwork.tile([120, BL], F32, tag="ds_t")
                    nc.vector.tensor_sub(ds_t[:, :], hs_aug[0:120, :], ns_t[:, :])
                    es_t = gwork.tile([120, BL], F32, tag="es_t")
                    nc.vector.tensor_mul(es_t[:, :], rzs[:, BL:2 * BL], ds_t[:, :])
                    nc.vector.tensor_add(hs_aug[0:120, :], ns_t[:, :], es_t[:, :])

                nc.sync.dma_start(dbg_hs_d, hs_aug[:])

                # =========== Stage 6: final linear (+ highway already in) ====
                nc.tensor.matmul(ps_fin[:, 0:M], h_aug[0:HIDR, :], w2a[:, :],
                                 start=False, stop=False, skip_group_check=True)
                nc.tensor.matmul(ps_fin[:, 0:M], hs_aug[:, :], w2b[:, :],
                                 start=False, stop=True, skip_group_check=True)
                out_sb = gwork.tile([BL, M], F32, tag="out_sb")
                nc.scalar.activation(out_sb[:, :], ps_fin[:, 0:M], AF.Sigmoid)
                nc.sync.dma_start(out_d, out_sb[:, :])

    nc.compile()
    return nc


def host_prep(inputs):
    """Build per-core input maps from the full model inputs."""
    x = np.asarray(inputs["x"], dtype=np.float32)
    conv_w = np.asarray(inputs["conv_w"], dtype=np.float32)
    conv_b = np.asarray(inputs["conv_b"], dtype=np.float32)

    x_flat = x.reshape(B, KC)
    xpad = np.zeros((B, KCP), np.float32)
    xpad[:, :KC] = x_flat
    xpad[:, KC] = 1.0
    # [kin, (ktile, b)]
    xt = np.ascontiguousarray(
        xpad.T.reshape(32, 128, B).transpose(1, 0, 2).reshape(128, 32 * B)
    )

    def gate(w, g, h):
        return w[g * h:(g + 1) * h]

    gWih, gWhh = np.asarray(inputs["gru1_Wih"], np.float32), np.asarray(
        inputs["gru1_Whh"], np.float32)
    gbih, gbhh = np.asarray(inputs["gru1_bih"], np.float32), np.asarray(
        inputs["gru1_bhh"], np.float32)
    sWih, sWhh = np.asarray(inputs["grus_Wih"], np.float32), np.asarray(
        inputs["grus_Whh"], np.float32)
    sbih, sbhh = np.asarray(inputs["grus_bih"], np.float32), np.asarray(
        inputs["grus_bhh"], np.float32)
    l1w, l1b = np.asarray(inputs["lin1_w"], np.float32), np.asarray(
        inputs["lin1_b"], np.float32)
    hww, hwb = np.asarray(inputs["hw_w"], np.float32), np.asarray(
        inputs["hw_b"], np.float32)

    def wh_g(g):
        m = np.zeros((HIDR + 1, HIDR), np.float32)
        m[:HIDR] = gate(gWhh, g, HIDR).T
        m[HIDR] = gate(gbhh, g, HIDR)
        return m

    def wx_g(g):
        m = np.zeros((HIDC + 1, HIDR), np.float32)
        m[:HIDC] = gate(gWih, g, HIDR).T
        m[HIDC] = gate(gbih, g, HIDR)
        return m

    def ls_g(g):
        m = np.zeros((121, 120), np.float32)
        w = gate(sWhh, g, HIDS)          # [5, 5]
        for k in range(SKIP):
            m[5 * k:5 * k + 5, 5 * k:5 * k + 5] = w.T
        m[120] = np.tile(gate(sbhh, g, HIDS), SKIP)
        return m

    wxs = np.zeros((HIDC + 1, 15), np.float32)
    wxs[:HIDC] = sWih.T            # [32, 15] gates (r, z, n) along columns
    wxs[HIDC] = sbih

    w2a = np.ascontiguousarray(l1w[:, :HIDR].T)           # [100, 24]
    w2b = np.zeros((121, M), np.float32)
    w2b[:120] = l1w[:, HIDR:].T                           # [120, 24]
    w2b[120] = l1b

    # highway: xh_full[w, m, b] ; per-core slice of b
    zt = x[:, P - HWIN:].reshape(B, HWIN, M)              # [b, w, m]
    xh_full = np.ascontiguousarray(zt.transpose(1, 2, 0))  # [w, m, b]
    hwv = np.concatenate([hww[0], hwb]).reshape(HWIN + 1, 1).astype(np.float32)

    hinit = np.zeros((HIDR + 1, BL), np.float32)
    hinit[HIDR] = 1.0
    hsinit = np.zeros((121, BL), np.float32)
    hsinit[120] = 1.0

    # conv weights per core, o-chunk-major 1MB tiles
    in_maps = []
    shared = dict(
        xt=xt,
        whr=wh_g(0), whz=wh_g(1), whn=wh_g(2),
        wxr=wx_g(0), wxz=wx_g(1), wxn=wx_g(2),
        lsr=ls_g(0), lsz=ls_g(1), lsn=ls_g(2),
        wxs=wxs, w2a=w2a, w2b=w2b, hwv=hwv,
        hinit=hinit, hsinit=hsinit,
    )
    for c in range(NC):
        wslice = conv_w.reshape(OC, KC)[c * OPC:(c + 1) * OPC]
        wtp = np.zeros((KCP, OPC), np.float32)
        wtp[:KC] = wslice.T
        wtp[KC] = conv_b[c * OPC:(c + 1) * OPC]
        wt = np.ascontiguousarray(
            wtp.reshape(8, 4, 128, 8, 504)
            .transpose(3, 0, 2, 1, 4)
            .reshape(64, 128, 2016)
        )
        xh_c = np.zeros((HWIN + 1, M * BL), np.float32)
        xh_c[:HWIN] = xh_full[:, :, c * BL:(c + 1) * BL].reshape(HWIN, M * BL)
        xh_c[HWIN] = 1.0
        in_maps.append(dict(shared, wt=wt, xh=xh_c))
    return in_maps


_CACHE = {}


def _get_program():
    if "nc" not in _CACHE:
        _CACHE["nc"] = build_program()
    return _CACHE["nc"]


def kernel(**inputs):
    nc = _get_program()
    in_maps = host_prep(inputs)
    res = run_bass_kernel_spmd(nc, in_maps, list(range(NC)))
    out = np.concatenate([res.results[i]["out"] for i in range(NC)], axis=0)
    return out.reshape(B, M1, M2, M3).astype(np.float32)
